# revision 13
# baseline (speedup 1.0000x reference)
"""LocationSensitiveAttention Trainium2 kernel (Bass/Tile), SPMD over 8 cores.

Contract: kernel(**inputs) takes FULL inputs (B=128 leading) and returns the
full (attention_context [B,512], attention_weights [B,1024]) tuple, matching
the reference. Sharding: data-parallel over batch, 16 examples per core.

Device-side math per example b:
  pq    = w_query.T @ hidden[b]                  (A=128 on partitions)
  conv  = w62.T @ im2col[b]                      (im2col built on host)
  loc   = w_loc.T @ conv                         (A x T)
  y     = tanh(loc + pmemT[b] + pq)              (ACT, bias=pq per-partition)
  e     = v.T @ y                                (1 x T rows -> (16,T) psum)
  w     = exp(e)  (no max-subtract: |e| <= ||v||_1 ~ 10, safe in fp32)
  ctx[b]= sum_t (w[t]/sum w) * memory[b,t,:]     (PE matmuls over T chunks)
Softmax normalization (1/sum) is folded into the final output copies.
"""

import os

import numpy as np

import concourse.bacc as bacc
import concourse.bass as bass
import concourse.mybir as mybir
import concourse.tile as tile
from concourse.bass_utils import run_bass_kernel_spmd

B, T = 128, 1024
RNN_DIM, EMB_DIM, ATT_DIM = 1024, 512, 128
N_FILT, KSIZE = 32, 31
PAD = KSIZE // 2
NCORES = 8
BPC = B // NCORES  # 16 examples per core
NTC = T // 128  # 8 T-chunks of 128
F32 = mybir.dt.float32

# Matmul operands stream at 1 col/cycle in fp16 (vs 4 cycles for fp32), and
# fp16 halves the dominant HBM traffic. PSUM accumulation stays fp32.
F16 = mybir.dt.float16


def build_kernel(nc: bass.Bass):
    """Declare DRAM tensors and emit the Tile program. Same program all cores."""
    # Per-core inputs (host-prepped layouts)
    mem_d = nc.dram_tensor("mem", [BPC, T, EMB_DIM], F16, kind="ExternalInput")
    pmt_d = nc.dram_tensor("pmt", [BPC, ATT_DIM, T], F16, kind="ExternalInput")
    ic_d = nc.dram_tensor("ic", [BPC, 2 * KSIZE, T], F16, kind="ExternalInput")
    ht_d = nc.dram_tensor("ht", [128, RNN_DIM // 128, BPC], F16, kind="ExternalInput")
    wq_d = nc.dram_tensor("wq", [128, RNN_DIM // 128, ATT_DIM], F16, kind="ExternalInput")
    w62_d = nc.dram_tensor("w62", [2 * KSIZE, N_FILT], F16, kind="ExternalInput")
    wl_d = nc.dram_tensor("wl", [N_FILT, ATT_DIM], F16, kind="ExternalInput")
    v_d = nc.dram_tensor("v", [ATT_DIM, 1], F16, kind="ExternalInput")
    mb_d = nc.dram_tensor("mb", [128, 128], F32, kind="ExternalInput")
    id128_d = nc.dram_tensor("id128", [128, 128], F32, kind="ExternalInput")
    blk_d = nc.dram_tensor("blk", [128, 128], F32, kind="ExternalInput")
    idh_d = nc.dram_tensor("idh", [128, 128], F16, kind="ExternalInput")
    # Outputs
    ctx_d = nc.dram_tensor("ctx", [BPC, EMB_DIM], F32, kind="ExternalOutput")
    attw_d = nc.dram_tensor("attw", [BPC, T], F32, kind="ExternalOutput")

    nrc = RNN_DIM // 128  # 8 rnn chunks

    with tile.TileContext(nc) as tc:
        with (
            tc.tile_pool(name="const", bufs=1) as const,
            tc.tile_pool(name="ic_sb", bufs=6) as ic_pool,
            tc.tile_pool(name="conv_sb", bufs=4) as conv_pool,
            tc.tile_pool(name="pm_sb", bufs=4) as pm_pool,
            tc.tile_pool(name="y_sb", bufs=4) as y_pool,
            tc.tile_pool(name="sm_sb", bufs=1) as sm_pool,
            tc.tile_pool(name="mem_sb", bufs=24) as mem_pool,
            tc.tile_pool(name="out_sb", bufs=2) as out_pool,
            tc.tile_pool(name="mm_ps", bufs=2, space="PSUM") as mm_ps,
            tc.tile_pool(name="small_ps", bufs=2, space="PSUM") as small_ps,
            tc.tile_pool(name="row_ps", bufs=3, space="PSUM") as row_ps,
            tc.tile_pool(name="ecol_ps", bufs=1, space="PSUM") as ecol_ps,
        ):
            # ---- constants ----
            w62_sb = const.tile([2 * KSIZE, N_FILT], F16)
            nc.sync.dma_start(out=w62_sb, in_=w62_d[:, :])
            wl_sb = const.tile([N_FILT, ATT_DIM], F16)
            nc.sync.dma_start(out=wl_sb, in_=wl_d[:, :])
            v_sb = const.tile([ATT_DIM, 1], F16)
            nc.sync.dma_start(out=v_sb, in_=v_d[:, :])
            id128_sb = const.tile([128, 128], F32)
            nc.sync.dma_start(out=id128_sb, in_=id128_d[:, :])
            blk_sb = const.tile([128, 128], F32)
            nc.sync.dma_start(out=blk_sb, in_=blk_d[:, :])
            idh_sb = const.tile([128, 128], F16)
            nc.sync.dma_start(out=idh_sb, in_=idh_d[:, :])
            mb_sb = const.tile([128, 128], F32)
            nc.sync.dma_start(out=mb_sb, in_=mb_d[:, :])
            ht_sb = const.tile([128, nrc, BPC], F16)
            nc.sync.dma_start(out=ht_sb, in_=ht_d[:, :, :])
            wq_sb = const.tile([128, nrc, ATT_DIM], F16)
            nc.sync.dma_start(out=wq_sb, in_=wq_d[:, :, :])

            # ---- pq = hidden @ w_query, laid out (A=128 part, BPC free) ----
            pq_ps = small_ps.tile([ATT_DIM, BPC], F32, tag="small")
            for c in range(nrc):
                nc.tensor.matmul(
                    pq_ps[:, :],
                    wq_sb[:, c, :],
                    ht_sb[:, c, :],
                    start=(c == 0),
                    stop=(c == nrc - 1),
                )
            pq_sb = const.tile([ATT_DIM, BPC], F32)
            nc.scalar.copy(out=pq_sb[:, :], in_=pq_ps[:, :])

            # ---- energies: e_cols[t', 8b+tc] = sum_a y[a, tc*128+t'] v[a] ----
            e_cols = ecol_ps.tile([128, 128], F32)
            for b in range(BPC):
                ic_sb = ic_pool.tile([2 * KSIZE, T], F16, tag="ic")
                nc.sync.dma_start(out=ic_sb, in_=ic_d[b, :, :])
                conv_sb = conv_pool.tile([N_FILT, T], F16, tag="conv")
                for h in range(2):
                    cps = mm_ps.tile([N_FILT, 512], F32, tag="mm")
                    nc.tensor.matmul(
                        cps[:, :],
                        w62_sb[:, :],
                        ic_sb[:, h * 512 : (h + 1) * 512],
                        start=True,
                        stop=True,
                    )
                    nc.scalar.copy(
                        out=conv_sb[:, h * 512 : (h + 1) * 512], in_=cps[:, :]
                    )
                pm_sb = pm_pool.tile([ATT_DIM, T], F16, tag="pm")
                nc.sync.dma_start(out=pm_sb, in_=pmt_d[b, :, :])
                y_sb = y_pool.tile([ATT_DIM, T], F16, tag="y")
                for h in range(2):
                    lps = mm_ps.tile([ATT_DIM, 512], F32, tag="mm")
                    # psum = loc = w_loc.T @ conv
                    nc.tensor.matmul(
                        lps[:, :],
                        wl_sb[:, :],
                        conv_sb[:, h * 512 : (h + 1) * 512],
                        start=True,
                        stop=False,
                    )
                    # psum += I.T @ pmem = pmem
                    nc.tensor.matmul(
                        lps[:, :],
                        idh_sb[:, :],
                        pm_sb[:, h * 512 : (h + 1) * 512],
                        start=False,
                        stop=True,
                    )
                    # y = tanh(loc + pmem + pq[b])
                    nc.scalar.activation(
                        out=y_sb[:, h * 512 : (h + 1) * 512],
                        in_=lps[:, :],
                        func=mybir.ActivationFunctionType.Tanh,
                        bias=pq_sb[:, b : b + 1],
                    )
                for tc_i in range(NTC):
                    col = b * NTC + tc_i
                    nc.tensor.matmul(
                        e_cols[:, col : col + 1],
                        y_sb[:, tc_i * 128 : (tc_i + 1) * 128],
                        v_sb[:, :],
                        start=True,
                        stop=True,
                    )

            # ---- softmax, batched; no max-subtract (|e| <= ||v||_1 ~ 10) ----
            ec_sb = sm_pool.tile([128, 128], F32, tag="ec")
            nc.scalar.copy(out=ec_sb[:, :], in_=e_cols[:, :])
            e2_ps = small_ps.tile([128, 128], F32, tag="small")
            nc.tensor.transpose(e2_ps[:, :], ec_sb[:, :], id128_sb[:, :])
            # mask bias add (mb2 is host-arranged to the (8b+tc, t') layout)
            exp_sb = sm_pool.tile([128, 128], F32, tag="exp")
            nc.vector.tensor_add(out=exp_sb[:, :], in0=mb_sb[:, :], in1=e2_ps[:, :])
            nc.scalar.activation(
                out=exp_sb[:, :],
                in_=exp_sb[:, :],
                func=mybir.ActivationFunctionType.Exp,
            )
            psums_sb = sm_pool.tile([128, 1], F32, tag="psums")
            nc.vector.tensor_reduce(
                out=psums_sb[:, :],
                in_=exp_sb[:, :],
                axis=mybir.AxisListType.X,
                op=mybir.AluOpType.add,
            )
            # blk2[p, m] = (p//8 == m//8): out[m] = sum over example group = s[m//8]
            s_ps = small_ps.tile([128, 1], F32, tag="small")
            nc.tensor.matmul(
                s_ps[:, :], blk_sb[:, :], psums_sb[:, :], start=True, stop=True
            )
            r_rep = sm_pool.tile([128, 1], F32, tag="rrep")
            nc.vector.reciprocal(out=r_rep[:, :], in_=s_ps[:, :])
            # normalized weights, still in (8b+tc, t') layout
            nc.scalar.activation(
                out=exp_sb[:, :],
                in_=exp_sb[:, :],
                func=mybir.ActivationFunctionType.Copy,
                scale=r_rep[:, 0:1],
            )
            # attention_weights output: one DMA, rows 8b+tc
            attw_rows = attw_d[:, :].rearrange("b (c t) -> (b c) t", c=NTC)
            nc.sync.dma_start(out=attw_rows, in_=exp_sb[:, :])

            # ---- transpose normalized weights -> wt2[t', 8b+tc] ----
            wt_ps = small_ps.tile([128, 128], F32, tag="small")
            nc.tensor.transpose(wt_ps[:, :], exp_sb[:, :], id128_sb[:, :])
            wt_sb = const.tile([128, 128], F16)
            nc.scalar.copy(out=wt_sb[:, :], in_=wt_ps[:, :])

            # ---- context: ctx[b] = sum_tc wt2[:, 8b+tc].T @ mem[b, tc] ----
            for b in range(BPC):
                cps = row_ps.tile([1, EMB_DIM], F32, tag="row")
                for tc_i in range(NTC):
                    m_sb = mem_pool.tile([128, EMB_DIM], F16, tag="mem")
                    nc.sync.dma_start(
                        out=m_sb, in_=mem_d[b, tc_i * 128 : (tc_i + 1) * 128, :]
                    )
                    col = b * NTC + tc_i
                    nc.tensor.matmul(
                        cps[:, :],
                        wt_sb[:, col : col + 1],
                        m_sb[:, :],
                        start=(tc_i == 0),
                        stop=(tc_i == NTC - 1),
                    )
                ctx_row = out_pool.tile([1, EMB_DIM], F32, tag="ctxr")
                nc.scalar.copy(out=ctx_row[:, :], in_=cps[:, :])
                nc.sync.dma_start(out=ctx_d[b : b + 1, :], in_=ctx_row[:, :])

    return nc


def prep_inputs(
    attention_hidden_state,
    memory,
    processed_memory,
    attention_weights_cat,
    mask,
    w_query,
    w_conv,
    w_loc,
    v,
):
    """Host-side layout prep -> per-core in_maps (list of dicts, core order)."""
    f32 = np.float32
    hid = np.ascontiguousarray(attention_hidden_state, dtype=f32)
    memory = np.ascontiguousarray(memory, dtype=f32)
    pmem = np.ascontiguousarray(processed_memory, dtype=f32)
    awc = np.ascontiguousarray(attention_weights_cat, dtype=f32)
    w_query = np.ascontiguousarray(w_query, dtype=f32)
    w_conv = np.ascontiguousarray(w_conv, dtype=f32)
    w_loc = np.ascontiguousarray(w_loc, dtype=f32)
    v = np.ascontiguousarray(v, dtype=f32)

    # shared (weight) tensors
    wq_arr = np.ascontiguousarray(
        w_query.reshape(RNN_DIM // 128, 128, ATT_DIM).transpose(1, 0, 2)
    ).astype(np.float16)
    w62 = np.ascontiguousarray(
        w_conv.transpose(1, 2, 0).reshape(2 * KSIZE, N_FILT)
    ).astype(np.float16)
    v_arr = np.ascontiguousarray(v.reshape(ATT_DIM, 1)).astype(np.float16)
    id128 = np.eye(128, dtype=f32)
    blk = np.zeros((128, 128), dtype=f32)
    for j in range(BPC):
        blk[j * NTC : (j + 1) * NTC, j * NTC : (j + 1) * NTC] = 1.0

    # im2col for the location conv (pad 15 both sides, kernel 31)
    xp = np.pad(awc, ((0, 0), (0, 0), (PAD, PAD)))
    ic_full = np.lib.stride_tricks.sliding_window_view(xp, T, axis=2)
    # ic_full: (B, 2, KSIZE, T); row (c,k) holds x[b,c,t+k-PAD]
    ic_full = np.ascontiguousarray(ic_full.reshape(B, 2 * KSIZE, T)).astype(np.float16)

    maskb = np.where(mask, f32(-1e30), f32(0.0)).astype(f32)
    pmt_full = np.ascontiguousarray(pmem.transpose(0, 2, 1)).astype(np.float16)
    idh = np.eye(128, dtype=np.float16)
    ht_full = np.ascontiguousarray(hid.T)  # (RNN, B)

    in_maps = []
    for c in range(NCORES):
        b0 = c * BPC
        ht_core = np.ascontiguousarray(
            ht_full[:, b0 : b0 + BPC].reshape(RNN_DIM // 128, 128, BPC).transpose(1, 0, 2)
        ).astype(np.float16)
        in_maps.append(
            {
                "mem": np.ascontiguousarray(memory[b0 : b0 + BPC]).astype(np.float16),
                "pmt": pmt_full[b0 : b0 + BPC],
                "ic": ic_full[b0 : b0 + BPC],
                "ht": ht_core,
                "wq": wq_arr,
                "w62": w62,
                "wl": w_loc.astype(np.float16),
                "v": v_arr,
                "mb": np.ascontiguousarray(
                    maskb[b0 : b0 + BPC].reshape(128, 128)
                ),
                "id128": id128,
                "blk": blk,
                "idh": idh,
            }
        )
    return in_maps


def make_nc():
    nc = bacc.Bacc(
        "TRN2",
        target_bir_lowering=False,
        debug=False,
        enable_asserts=False,
        num_devices=NCORES,
    )
    build_kernel(nc)
    nc.compile()
    return nc


def kernel(
    attention_hidden_state,
    memory,
    processed_memory,
    attention_weights_cat,
    mask,
    w_query,
    w_conv,
    w_loc,
    v,
    _trace=False,
    _trace_kwargs=None,
):
    in_maps = prep_inputs(
        attention_hidden_state,
        memory,
        processed_memory,
        attention_weights_cat,
        mask,
        w_query,
        w_conv,
        w_loc,
        v,
    )
    nc = make_nc()
    res = run_bass_kernel_spmd(
        nc,
        in_maps,
        core_ids=list(range(NCORES)),
        trace=_trace,
        **(_trace_kwargs or {}),
    )
    ctx = np.concatenate([r["ctx"] for r in res.results], axis=0)
    attw = np.concatenate([r["attw"] for r in res.results], axis=0)
    kernel.last_result = res
    return ctx, attw
